# revision 2
# baseline (speedup 1.0000x reference)
"""LocalExpansion (7x7 unfold) Trainium2 Bass kernel, v2: column-sharded.

Full input x: [2, 8, 2304, 64] f32 (16 images of 48x48x64).
Full output:  [2, 8, 2304, 49, 64] f32 = [768 p-rows, 48 tok, 3136 f]
where p-row R = (b*8+h)*48 + y holds tokens (y, x=0..47).

Sharding: core c owns token-columns x in [6c, 6c+6) of ALL 768 p-rows and
writes a compact per-core [768, 18816] f32 buffer — a fully dense
partition-major DMA stream (128-partition, the fastest measured HBM
write pattern). The host interleaves columns afterward.

Per-core pipeline over 6 bands of 128 rows x 7 filter-rows (42 rounds):
  - host inputs: xc[16, 48, 768] f32 (12-col window 6c-3..6c+8, zero
    padded) and mmat[128, 84*128] bf16 (0/1 shift-diagonal stationaries)
  - Pool(SWDGE): cast-loads images to bf16 pads (image m at partitions
    64*(m%2)+3..50, col-slot m//2)
  - PE round r=(g,i): 6 matmuls (2 pad col-slots x 3 256-f chunks)
    accumulate the shifted padded rows: psum[r%3][q, w*64+d] =
    img[m(q), y(q)+i-3, 6c+w-3, d]  (zeros at image borders)
  - copies (engine r%3 in {DVE, Act, Pool}): partition-local gather
    tok[p, (g%2)*18816 + xl*3136 + i*448 + j*64 + d] =
        psum[p, (xl+j)*64 + d]
    (each psum tensor has exactly ONE reader engine: two engines
    concurrently reading one PSUM tensor faults on HW)
  - per band: two half DMAs [[18816,128],[1,9408]] on sync+scalar rings.

bf16 rounding of inputs gives rel err ~3e-3, well under the 2e-2 gate.
"""

import numpy as np
import ml_dtypes

PADSW = 8 * 768          # pads cols per partition (8 slots x 768 f bf16)
TOKW = 2 * 18816         # tok cols per partition (2 band slots, f32)
MTW = 84 * 128           # stationary tiles (84 x 128 bf16)
N_CORES = 8

# piece tables: per band g, list of (m, qa, qb); delta = 64*(m%2)+128*g-48*m+i
_PIECES = {
    0: [(0, 0, 48), (1, 48, 96), (2, 96, 128)],
    1: [(2, 0, 16), (3, 16, 64), (4, 64, 112), (5, 112, 128)],
    2: [(5, 0, 32), (6, 32, 80), (7, 80, 128)],
    3: [(8, 0, 48), (9, 48, 96), (10, 96, 128)],
    4: [(10, 0, 16), (11, 16, 64), (12, 64, 112), (13, 112, 128)],
    5: [(13, 0, 32), (14, 32, 80), (15, 80, 128)],
}
_MAXIMG = {0: 2, 1: 5, 2: 7, 3: 10, 4: 13, 5: 15}

_CACHE = {}


def _cdone(r):
    """(engine_index, count) for 'copy of round r completed'."""
    return r % 2, r // 2 + 1


def _build_nc(NB=6):
    import concourse.bass as bass
    import concourse.mybir as mybir

    f32 = mybir.dt.float32
    bf16 = mybir.dt.bfloat16

    nc = bass.Bass(trn_type="TRN2")
    xc = nc.dram_tensor("xc", [16, 48, 768], f32, kind="ExternalInput")
    mmat = nc.dram_tensor("mmat", [128, MTW], bf16, kind="ExternalInput")
    out = nc.dram_tensor("out", [768, 18816], f32, kind="ExternalOutput")

    with (
        nc.sbuf_tensor("pads", [128, PADSW], bf16) as pads,
        nc.sbuf_tensor("tok", [128, TOKW], f32) as tok,
        nc.sbuf_tensor("mt", [128, MTW], bf16) as mt,
        nc.psum_tensor("ps0", [128, 768], f32) as ps0,
        nc.psum_tensor("ps1", [128, 768], f32) as ps1,
        nc.semaphore("mld") as mld,
        nc.semaphore("mz") as mz,
        nc.semaphore("ld") as ld,
        nc.semaphore("pdone") as pdone,
        nc.semaphore("cdv") as cdv,
        nc.semaphore("cac") as cac,
        nc.semaphore("ws0") as ws0,
        nc.semaphore("ws1") as ws1,
    ):
        ps = (ps0, ps1)
        csem = (cdv, cac)

        # --- sync ring: load stationaries
        nc.sync.dma_start(
            out=bass.AP(mt, 0, [[MTW, 128], [1, MTW]]),
            in_=bass.AP(mmat, 0, [[MTW, 128], [1, MTW]]),
        ).then_inc(mld, 16)

        # --- DVE: zero the pads region (borders stay zero forever)
        nc.vector.memset(
            bass.AP(pads, 0, [[PADSW, 128], [1, PADSW]]), 0.0
        ).then_inc(mz, 1)

        # --- Pool: image cast-loads (interleaved with its copies below)
        def load_img(m):
            nc.gpsimd.dma_start(
                out=bass.AP(
                    pads,
                    (64 * (m % 2) + 3) * PADSW + (m // 2) * 768,
                    [[PADSW, 48], [1, 768]],
                ),
                in_=bass.AP(xc, m * 48 * 768, [[768, 48], [1, 768]]),
            ).then_inc(ld, 16)

        nc.gpsimd.wait_ge(mz, 1)
        for m in range(16):
            load_img(m)

        # --- PE: 6 matmuls per round r
        for r in range(7 * NB):
            g, i = r // 7, r % 7
            pieces = _PIECES[g]
            slots = sorted(set(m // 2 for (m, _, _) in pieces))
            if r == 0:
                nc.tensor.wait_ge(mld, 16)
            nc.tensor.wait_ge(ld, 16 * (_MAXIMG[g] + 1))
            if r >= 2:
                e, cnt = _cdone(r - 2)
                nc.tensor.wait_ge(csem[e], cnt)
            pt = ps[r % 2]
            for k in range(3):
                for half, sl in enumerate(slots):
                    t = 2 * r + half
                    nc.tensor.matmul(
                        bass.AP(pt, k * 256, [[768, 128], [1, 256]]),
                        bass.AP(mt, t * 128, [[MTW, 128], [1, 128]]),
                        bass.AP(pads, sl * 768 + k * 256,
                                [[PADSW, 128], [1, 256]]),
                        start=(half == 0),
                        stop=(half == 1),
                        skip_group_check=True,
                    ).then_inc(pdone, 1)

        # --- copies: engine r%3; band DMAs split across both rings
        for r in range(7 * NB):
            g, i = r // 7, r % 7
            e = r % 2
            pt = ps[r % 2]
            so = (g % 2) * 18816
            eng = (nc.vector, nc.scalar)[e]
            if i == 0 and g >= 2:
                for en in (nc.vector, nc.scalar):
                    en.wait_ge(ws0, 16 * (g - 1))
                    en.wait_ge(ws1, 16 * (g - 1))
            eng.wait_ge(pdone, 6 * (r + 1))
            if e == 1:
                eng.copy(
                    bass.AP(tok, so + i * 448,
                            [[TOKW, 128], [3136, 6], [1, 448]]),
                    bass.AP(pt, 0, [[768, 128], [64, 6], [1, 448]]),
                ).then_inc(csem[e], 1)
            else:
                eng.tensor_scalar_add(
                    bass.AP(tok, so + i * 448,
                            [[TOKW, 128], [3136, 6], [1, 448]]),
                    bass.AP(pt, 0, [[768, 128], [64, 6], [1, 448]]),
                    0.0,
                ).then_inc(csem[e], 1)
            # band writes: two half DMAs on the two rings
            if i == 6:
                cnts = [sum(1 for rr in range(7 * g + 7) if rr % 2 == ee)
                        for ee in range(2)]
                for ee in range(2):
                    nc.sync.wait_ge(csem[ee], cnts[ee])
                    nc.scalar.wait_ge(csem[ee], cnts[ee])
                nc.sync.dma_start(
                    out=bass.AP(out, g * 128 * 18816,
                                [[18816, 128], [1, 9408]]),
                    in_=bass.AP(tok, so, [[TOKW, 128], [1, 9408]]),
                ).then_inc(ws0, 16)
                nc.scalar.dma_start(
                    out=bass.AP(out, g * 128 * 18816 + 9408,
                                [[18816, 128], [1, 9408]]),
                    in_=bass.AP(tok, so + 9408, [[TOKW, 128], [1, 9408]]),
                ).then_inc(ws1, 16)

        nc.sync.wait_ge(ws0, 16 * NB)
        nc.sync.wait_ge(ws1, 16 * NB)
        nc.scalar.wait_ge(ws0, 16 * NB)
        nc.scalar.wait_ge(ws1, 16 * NB)
    return nc


def _build_mmat():
    """84 stationary tiles: t = 2r+half, M[s, t*128+q] = 1 iff the piece
    covering q in (band r//7, slot-half) maps out-partition q from
    src-partition s = q + delta."""
    M = np.zeros((128, 84 * 128), np.float32)
    for r in range(42):
        g, i = r // 7, r % 7
        pieces = _PIECES[g]
        slots = sorted(set(m // 2 for (m, _, _) in pieces))
        for half, sl in enumerate(slots):
            t = 2 * r + half
            for (m, qa, qb) in pieces:
                if m // 2 != sl:
                    continue
                delta = 64 * (m % 2) + 128 * g - 48 * m + i
                for q in range(qa, qb):
                    M[q + delta, t * 128 + q] = 1.0
    return M.astype(ml_dtypes.bfloat16)


_MMAT = None


def _host_inputs(x):
    global _MMAT
    if _MMAT is None:
        _MMAT = _build_mmat()
    xi = np.ascontiguousarray(x.reshape(16, 48, 48, 64))
    in_maps = []
    for c in range(N_CORES):
        win = np.zeros((16, 48, 12, 64), np.float32)
        lo = 6 * c - 3
        s0, s1 = max(0, lo), min(48, lo + 12)
        win[:, :, s0 - lo:s1 - lo, :] = xi[:, :, s0:s1, :]
        in_maps.append({"xc": np.ascontiguousarray(win.reshape(16, 48, 768)),
                        "mmat": _MMAT})
    return in_maps


def kernel(x, height=48, width=48):
    from concourse.bass_utils import run_bass_kernel_spmd

    x = np.asarray(x)
    in_maps = _host_inputs(x)
    if "nc" not in _CACHE:
        _CACHE["nc"] = _build_nc()
    res = run_bass_kernel_spmd(_CACHE["nc"], in_maps,
                               core_ids=list(range(N_CORES)))
    full = np.empty((768, 48, 3136), np.float32)
    for c in range(N_CORES):
        full[:, 6 * c:6 * c + 6, :] = res.results[c]["out"].reshape(768, 6, 3136)
    return full.reshape(2, 8, 2304, 49, 64)
